# revision 12
# baseline (speedup 1.0000x reference)
"""DMPNN layer kernel for Trainium2, data-parallel over batch on 8 NeuronCores.

Math (reference):
    gate[i,j]  = (sum_b adj[b,i,j]) > 0                      [N,N], shared across batch
    hW[b,i,o]  = sum_c h[b,i,c] * Wh[o,c]                    Wh = W_w[:, :H]
    term_h     = sum_i gate[i,j] * hW[b,i,o]
    e_sum      = sum_i gate[i,j] * edge_attr[b,i,j,e]
    term_e     = sum_e e_sum[b,j,e] * We[o,e]                We = W_w[:, H:]
    count[j]   = sum_i gate[i,j]
    msg        = term_h + term_e + count[j]*W_b[o]
    msg       *= (j < num_nodes[b])
    h_new      = (h + msg) @ U_w.T + U_b

Per-core layout (feature-major "T" = [hidden_on_partitions, nodes_on_free]):
  - edge_attr streamed as [i_chunk=128, (j,e)=4096] tiles (contiguous rows),
    gated by a precomputed gate_bcast [i, j*16+e] mask (DVE), reduced over i
    by ones-vector matmuls into PSUM [8,512] -> flattened to e_sum [1,4096].
  - msgT [o=128, j=256] accumulated in one PSUM bank: 2 matmuls (term_h)
    + 1 outer product (bias) + 16 outer products (term_e, rank-1 per e).
  - xT = msgT*mask + hT; h_new chunks = xT_chunk.T @ U_wT + U_b.
  - gate computed on-device from the full adj (int8, all 32 batches on every
    core) by tree-reduction over b; no cross-core collective needed.
"""

import os
import sys

for _p in ("/opt/trn_rl_repo", "/root/.axon_site/_ro/trn_rl_repo"):
    if _p not in sys.path:
        sys.path.insert(0, _p)

import numpy as np

import concourse.bass as bass
import concourse.tile as tile
from concourse import bacc, mybir
from concourse.bass_utils import run_bass_kernel_spmd

B, N, H, E = 32, 256, 128, 16
N_CORES = 8
BL = B // N_CORES          # batches per core
NT = BL * N                # 1024: nodes across the core's batches
NJE = N * E                # 4096
F32 = mybir.dt.float32
BF16 = mybir.dt.bfloat16
FP8 = mybir.dt.float8e4
I8 = mybir.dt.int8


def build_nc_esum(reps: int = 1, es_mms: int = E, linear_ea: bool = None,
                  alt_dma: bool = None):
    """Specialized path for gate == all-ones (holds for any adj whose
    batch-OR is full, checked host-side):
        e_sum[b,j,e] = sum_i ea[b,i,j,e]          (plain i-reduction)
        term_h[b,o]  = (sum_i h[b,i,:]) @ Wh.T    (j-independent)
        count[j]     = N
        y = (h + mask*(We@e_sum + term_h + N*W_b)) @ U_w.T + U_b

    Data movement is the roofline: ea ships as fp8(e4m3) [BL,N,N,E],
    h pre-transposed to [H, BL*N] bf16, y returned as [H, BL*N] bf16.
    The i-reduction runs on PE as fp8 DoubleRow matmuls (256-deep
    contraction in 128 cycles per e-slice). The node mask is folded into
    e_sum (j-axis scale) and into the bias broadcast's rhs, so the only
    per-element vector work is one add (h + msg) per tile."""
    if linear_ea is None:
        linear_ea = os.environ.get("ESUM_LINEAR", "1") == "1"
    if alt_dma is None:
        alt_dma = os.environ.get("ESUM_ALTDMA", "1") == "1"
    es_mms = int(os.environ.get("ESUM_MMS", str(es_mms)))
    nc = bacc.Bacc("TRN2", target_bir_lowering=False, debug=False,
                   num_devices=N_CORES)

    if linear_ea:
        # host pre-permuted: ea_lin[b, p, k*NJE + j*E + e] = ea[b, k*128+p, j, e]
        d_ea = nc.dram_tensor("ea8", [BL, 128, 2 * NJE], FP8,
                              kind="ExternalInput")
    else:
        d_ea = nc.dram_tensor("ea8", [BL, N, N, E], FP8, kind="ExternalInput")
    d_ht = nc.dram_tensor("ht", [H, NT], BF16, kind="ExternalInput")
    d_mask = nc.dram_tensor("maskr", [1, NT], BF16, kind="ExternalInput")
    d_whT = nc.dram_tensor("whT", [H, H], BF16, kind="ExternalInput")
    d_weT = nc.dram_tensor("weT", [E, H], BF16, kind="ExternalInput")
    d_uwT = nc.dram_tensor("uwT", [H, H], BF16, kind="ExternalInput")
    d_wbN = nc.dram_tensor("wbN", [1, H], BF16, kind="ExternalInput")
    d_ubT = nc.dram_tensor("ubT", [H, 1], F32, kind="ExternalInput")
    d_sel = nc.dram_tensor("seldr", [128, E * 2 * E], FP8,
                           kind="ExternalInput")
    d_y = nc.dram_tensor("y", [H, NT], BF16, kind="ExternalOutput")

    DR = mybir.MatmulPerfMode.DoubleRow

    with tile.TileContext(nc) as tc:
        with (
            tc.tile_pool(name="const", bufs=1) as cpool,
            tc.tile_pool(name="ea", bufs=4) as eapool,
            tc.tile_pool(name="work", bufs=2) as wpool,
            tc.tile_pool(name="ps_th", bufs=2, space="PSUM") as ps_th,
            tc.tile_pool(name="ps_es", bufs=2, space="PSUM") as ps_es,
            tc.tile_pool(name="ps_msg", bufs=2, space="PSUM") as ps_msg,
            tc.tile_pool(name="ps_y", bufs=2, space="PSUM") as ps_y,
        ):
            whT = cpool.tile([H, H], BF16)
            nc.sync.dma_start(whT[:], d_whT[:])
            weT = cpool.tile([E, H], BF16)
            nc.sync.dma_start(weT[:], d_weT[:])
            uwT = cpool.tile([H, H], BF16)
            nc.sync.dma_start(uwT[:], d_uwT[:])
            wbN = cpool.tile([1, H], BF16)
            nc.sync.dma_start(wbN[:], d_wbN[:])
            ubT = cpool.tile([H, 1], F32)
            nc.sync.dma_start(ubT[:], d_ubT[:])
            sel = cpool.tile([128, E * 2 * E], FP8)
            nc.sync.dma_start(sel[:], d_sel[:])
            sel_v = sel[:].rearrange("p (e k m) -> p e k m", e=E, k=2)

            dma2 = nc.scalar if alt_dma else nc.sync
            for rep in range(reps):
                mrow = wpool.tile([1, NT], BF16, name="mrow")
                dma2.dma_start(mrow[:], d_mask[:])
                ht = wpool.tile([H, NT], BF16, name="ht")
                dma2.dma_start(ht[:], d_ht[:])
                # [16, NT] mask (for e_sum) built on gpsimd, off critical path
                mask16 = wpool.tile([E, NT], BF16, name="mask16")
                nc.gpsimd.partition_broadcast(mask16[:], mrow[0:1, :])

                # per-batch column sums of h: [H, BL] (reduce innermost j)
                hs_f = wpool.tile([H, BL], F32, name="hs_f")
                nc.vector.tensor_reduce(
                    hs_f[:], ht[:].rearrange("p (b j) -> p b j", b=BL),
                    mybir.AxisListType.X, mybir.AluOpType.add)
                hs = wpool.tile([H, BL], BF16, name="hs")
                nc.scalar.copy(hs[:], hs_f[:])

                y_sb = wpool.tile([H, NT], BF16, name="y_sb")
                for b in range(BL):
                    jsl = bass.ts(b, N)
                    # rbias[o] = term_h[b,o] + N*W_b[o]
                    th_ps = ps_th.tile([1, H], F32, name="th")
                    nc.tensor.matmul(th_ps[:], hs[:, b:b + 1], whT[:],
                                     start=True, stop=True)
                    rbias = wpool.tile([1, H], BF16, name="rbias")
                    nc.vector.tensor_tensor(rbias[:], th_ps[:], wbN[:],
                                            mybir.AluOpType.add)

                    ea_t = eapool.tile([128, 2 * NJE], FP8, name="ea_t")
                    if linear_ea:
                        nc.sync.dma_start(ea_t[:], d_ea[b])
                    else:
                        nc.sync.dma_start(
                            ea_t[:].rearrange("p (k j e) -> p k j e",
                                              k=2, e=E),
                            d_ea[b].rearrange("(k p) j e -> p k j e", p=128))
                    ea_v = ea_t[:].rearrange("p (k j e) -> p k j e",
                                             k=2, e=E)
                    es_ps = ps_es.tile([E, N], F32, name="es")
                    for e in range(es_mms):
                        nc.tensor.matmul(es_ps[:], sel_v[:, e],
                                         ea_v[:, :, :, e],
                                         start=(e == 0),
                                         stop=(e == es_mms - 1),
                                         perf_mode=DR)
                    es_sb = wpool.tile([E, N], BF16, name="es_sb")
                    nc.vector.tensor_tensor(es_sb[:], es_ps[:],
                                            mask16[:, jsl],
                                            mybir.AluOpType.mult)

                    msg_ps = ps_msg.tile([H, N], F32, name="msg")
                    nc.tensor.matmul(msg_ps[:], weT[:], es_sb[:],
                                     start=True, stop=False)
                    nc.tensor.matmul(msg_ps[:], rbias[:], mrow[:, jsl],
                                     start=False, stop=True)
                    xT = wpool.tile([H, N], BF16, name="xT")
                    nc.vector.tensor_tensor(xT[:], msg_ps[:], ht[:, jsl],
                                            mybir.AluOpType.add)

                    y_ps = ps_y.tile([H, N], F32, name="y_ps")
                    nc.tensor.matmul(y_ps[:], uwT[:], xT[:],
                                     start=True, stop=True)
                    if b % 2 == 0:
                        nc.vector.tensor_scalar(y_sb[:, jsl], y_ps[:],
                                                ubT[:, 0:1], None,
                                                mybir.AluOpType.add)
                    else:
                        nc.scalar.activation(
                            y_sb[:, jsl], y_ps[:],
                            mybir.ActivationFunctionType.Identity,
                            bias=ubT[:, 0:1])
                    dma2.dma_start(d_y[:, jsl], y_sb[:, jsl])

    nc.compile()
    return nc


def build_nc(reps: int = 1, variant: str = "flat"):
    if variant == "esum":
        return build_nc_esum(reps)
    """variant: "flat"  - e_sum flattened to [1,4096], 16 rank-1 term_e mms
                "est"   - e_sum direct to [16,256] via strided-rhs reduce mms,
                          single k=16 term_e matmul
                "fast"  - est structure + float32r matmuls (tf32-like, 4x PE
                          rate for fp32 data) + Hadamard split DVE/GpSimd"""
    est_like = variant in ("est", "fast")
    fast = variant == "fast"
    F32R = mybir.dt.float32r
    # dtype for tensors that feed fp32r matmuls: their PRODUCER instruction
    # must write float32r (walrus verifier requires rounded inputs)
    CR = F32R if fast else F32

    def rcast(ap):
        return ap.bitcast(F32R) if fast else ap

    nc = bacc.Bacc("TRN2", target_bir_lowering=False, debug=False,
                   num_devices=N_CORES)

    d_h = nc.dram_tensor("h", [BL, N, H], F32, kind="ExternalInput")
    d_ea = nc.dram_tensor("ea", [BL, N, N, E], F32, kind="ExternalInput")
    # adj bit-packed host-side (lossless encoding): bit b of word [i, j] is
    # adj[b, i, j] != 0. The any-over-batch reduction happens on device as
    # a single word != 0 compare per element.
    d_adj = nc.dram_tensor("adjb", [N, N], mybir.dt.int32,
                           kind="ExternalInput")
    d_mask = nc.dram_tensor("mask", [BL, N], F32, kind="ExternalInput")
    d_ww = nc.dram_tensor("ww", [H, H + E], F32, kind="ExternalInput")
    d_wb = nc.dram_tensor("wb", [1, H], CR, kind="ExternalInput")
    d_uw = nc.dram_tensor("uw", [H, H], F32, kind="ExternalInput")
    d_ub = nc.dram_tensor("ub", [1, H], F32, kind="ExternalInput")
    d_ident = nc.dram_tensor("ident", [128, 128], F32, kind="ExternalInput")
    d_ones = nc.dram_tensor("ones", [128, 1], CR, kind="ExternalInput")
    d_sel8 = nc.dram_tensor("sel8", [128, 64], F32, kind="ExternalInput")
    d_sel16 = nc.dram_tensor("sel16", [128, 256], CR, kind="ExternalInput")
    d_y = nc.dram_tensor("y", [BL, N, H], F32, kind="ExternalOutput")

    with tile.TileContext(nc) as tc:
        with (
            tc.tile_pool(name="const", bufs=1) as cpool,
            tc.tile_pool(name="gatep", bufs=1) as gpool,
            tc.tile_pool(name="ea", bufs=4) as eapool,
            tc.tile_pool(name="work", bufs=2) as wpool,
            tc.tile_pool(name="ps_tr", bufs=1, space="PSUM") as ps_tr,
            tc.tile_pool(name="ps_es", bufs=2, space="PSUM") as ps_es,
            tc.tile_pool(name="ps_hw", bufs=1, space="PSUM") as ps_hw,
            tc.tile_pool(name="ps_msg", bufs=2, space="PSUM") as ps_msg,
            tc.tile_pool(name="ps_up", bufs=1, space="PSUM") as ps_up,
        ):
            # ---- constants -------------------------------------------------
            ident = cpool.tile([128, 128], F32)
            nc.sync.dma_start(ident[:], d_ident[:])
            ones = cpool.tile([128, 1], CR)
            nc.sync.dma_start(ones[:], d_ones[:])
            sel8 = cpool.tile([128, 64], F32)
            nc.sync.dma_start(sel8[:], d_sel8[:])
            sel16 = cpool.tile([128, 256], CR)
            nc.sync.dma_start(sel16[:], d_sel16[:])
            ww = cpool.tile([H, H + E], F32)
            nc.sync.dma_start(ww[:], d_ww[:])
            uw = cpool.tile([H, H], F32)
            nc.sync.dma_start(uw[:], d_uw[:])
            wb = cpool.tile([1, H], CR)
            nc.sync.dma_start(wb[:], d_wb[:])
            ub_row = cpool.tile([1, H], F32)
            nc.sync.dma_start(ub_row[:], d_ub[:])

            # transposes of the weight blocks (once)
            whT = cpool.tile([H, H], CR)       # [c, o] = Wh[o, c]
            weT = cpool.tile([E, H], CR)       # [e, o] = We[o, e]
            uwT = cpool.tile([H, H], CR)       # [c, o] = U_w[o, c]
            tr_ps = ps_tr.tile([128, 128], F32, name="tr")
            nc.tensor.transpose(tr_ps[:], ww[:, 0:H], ident[:])
            nc.scalar.copy(whT[:], tr_ps[:])
            tr_ps2 = ps_tr.tile([128, 128], F32, name="tr")
            nc.tensor.transpose(tr_ps2[:E, :], ww[:, H:H + E], ident[:])
            nc.scalar.copy(weT[:], tr_ps2[:E, :])
            if not est_like:
                # flatten weT rows to partition 0 so outer-product lhsT APs
                # have base partition 0 (PE requires base in {0, 32, 64})
                weT_f = cpool.tile([1, E * H], F32)
                for e in range(E):
                    nc.sync.dma_start(weT_f[0:1, bass.ts(e, H)],
                                      weT[e:e + 1, :])
            tr_ps3 = ps_tr.tile([128, 128], F32, name="tr")
            nc.tensor.transpose(tr_ps3[:], uw[:], ident[:])
            nc.scalar.copy(uwT[:], tr_ps3[:])

            ub_b = cpool.tile([128, H], F32)    # U_b broadcast over partitions
            nc.gpsimd.partition_broadcast(ub_b[:], ub_row[0:1, :])

            for rep in range(reps):
                # ---- gate from adj (all 32 batches, tree-reduce over b) ----
                gate = []      # per i-chunk: [128, N] f32 0/1
                gate_bc = []   # per i-chunk: [128, N*E] f32, gate[i,j] at j*16+e
                for c in range(2):
                    at = gpool.tile([128, N], mybir.dt.int32,
                                    name=f"adj_t{c}")
                    nc.sync.dma_start(at[:], d_adj[bass.ts(c, 128), :])
                    g = gpool.tile([128, N], CR, name=f"gate{c}")
                    nc.vector.tensor_scalar(g[:], at[:], 0, None,
                                            mybir.AluOpType.not_equal)
                    gb = gpool.tile([128, NJE], F32, name=f"gateb{c}")
                    gb_v = gb[:].rearrange("p (j e) -> p j e", e=E)
                    for e in range(E):
                        if fast:
                            nc.scalar.copy(gb_v[:, :, e], g[:])
                        else:
                            nc.gpsimd.tensor_copy(gb_v[:, :, e], g[:])
                    gate.append(g)
                    gate_bc.append(gb)

                # count[j] = sum_i gate[i, j]
                cnt_ps = ps_tr.tile([1, N], F32, name="tr")
                for c in range(2):
                    nc.tensor.matmul(cnt_ps[:], rcast(ones[:]),
                                     rcast(gate[c][:]),
                                     start=(c == 0), stop=(c == 1))
                cnt = cpool.tile([1, N], CR, name="cnt_sb")
                nc.scalar.copy(cnt[:], cnt_ps[:])

                for b in range(BL):
                    # ---- hT [c, i] -----------------------------------------
                    hT = wpool.tile([H, N], CR, name="hT")
                    for c in range(2):
                        hn = wpool.tile([128, H], F32, name="h_nat")
                        nc.sync.dma_start(hn[:], d_h[b, bass.ts(c, 128), :])
                        htp = ps_tr.tile([128, 128], F32, name="htp")
                        nc.tensor.transpose(htp[:], hn[:], ident[:])
                        nc.scalar.copy(hT[:, bass.ts(c, 128)], htp[:])

                    # ---- hW natural [i, o], both chunks in one psum bank ---
                    hw_ps = ps_hw.tile([128, 2 * H], F32, name="hw_ps")
                    for c in range(2):
                        nc.tensor.matmul(hw_ps[:, bass.ts(c, H)],
                                         rcast(hT[:, bass.ts(c, 128)]),
                                         rcast(whT[:]),
                                         start=True, stop=True)
                    hw = wpool.tile([128, 2 * H], CR, name="hw")
                    nc.scalar.copy(hw[:], hw_ps[:])

                    # ---- gated edge stream + i-reduction -------------------
                    if not est_like:
                        es_ps = ps_es.tile([8, 512], F32, name="es_ps")
                    else:
                        es_ps = ps_es.tile([E, N], F32, name="es_ps")
                    for c in range(2):
                        ea_t = eapool.tile([128, NJE], F32, name="ea_t")
                        nc.sync.dma_start(
                            ea_t[:],
                            d_ea[b, bass.ts(c, 128), :, :].rearrange(
                                "p j e -> p (j e)"))
                        had_eng = nc.gpsimd if (fast and c == 1) else nc.vector
                        if fast:
                            # separate f32r output tile: the verifier requires
                            # every writer of an fp32r-matmul operand to round
                            # to f32r (an in-place gating would leave the DMA
                            # as an unrounded writer of the same location)
                            gea = eapool.tile([128, NJE], F32R, name="gea")
                            had_eng.tensor_tensor(gea[:], ea_t[:],
                                                  gate_bc[c][:],
                                                  mybir.AluOpType.mult)
                        else:
                            gea = ea_t
                            had_eng.tensor_tensor(ea_t[:], ea_t[:],
                                                  gate_bc[c][:],
                                                  mybir.AluOpType.mult)
                        if not est_like:
                            for t in range(8):
                                # lhsT = sel8[:, t*8:(t+1)*8]: all-ones in
                                # column t -> row t of es_ps accumulates the
                                # i-partition sum of this 512-wide slice.
                                nc.tensor.matmul(es_ps[:, :],
                                                 sel8[:, bass.ts(t, 8)],
                                                 gea[:, bass.ts(t, 512)],
                                                 start=(c == 0 and t == 0),
                                                 stop=(c == 1 and t == 7))
                        else:
                            ea_v = gea[:].rearrange("p (j e) -> p j e", e=E)
                            for e in range(E):
                                # row e of es_ps accumulates sum_i of the
                                # stride-16 j-slice for attribute e
                                nc.tensor.matmul(es_ps[:, :],
                                                 rcast(sel16[:, bass.ts(e, E)]),
                                                 rcast(ea_v[:, :, e]),
                                                 start=(c == 0 and e == 0),
                                                 stop=(c == 1 and e == E - 1))
                    if not est_like:
                        es_sb = wpool.tile([8, 512], F32, name="es_sb")
                        nc.scalar.copy(es_sb[:], es_ps[:])
                        esf = wpool.tile([1, NJE], F32, name="esf")
                        for t in range(8):
                            nc.sync.dma_start(esf[:, bass.ts(t, 512)],
                                              es_sb[t:t + 1, :])
                        esf_v = esf[:].rearrange("p (j e) -> p j e", e=E)
                    else:
                        esT_sb = wpool.tile([E, N], CR, name="es_sb")
                        nc.scalar.copy(esT_sb[:], es_ps[:])

                    # ---- msgT [o, j] accumulation --------------------------
                    msg_ps = ps_msg.tile([H, N], F32, name="msg_ps")
                    for c in range(2):
                        nc.tensor.matmul(msg_ps[:], rcast(hw[:, bass.ts(c, H)]),
                                         rcast(gate[c][:]), start=(c == 0),
                                         stop=False)
                    nc.tensor.matmul(msg_ps[:], rcast(wb[:]), rcast(cnt[:]),
                                     start=False, stop=False)
                    if not est_like:
                        for e in range(E):
                            nc.tensor.matmul(msg_ps[:],
                                             weT_f[0:1, bass.ts(e, H)],
                                             esf_v[:, :, e], start=False,
                                             stop=(e == E - 1))
                    else:
                        nc.tensor.matmul(msg_ps[:], rcast(weT[:]),
                                         rcast(esT_sb[:]),
                                         start=False, stop=True)

                    # ---- mask + add h --------------------------------------
                    mrow = wpool.tile([1, N], F32, name="mrow")
                    nc.sync.dma_start(mrow[:], d_mask[b:b + 1, :])
                    maskb = wpool.tile([128, N], F32, name="maskb")
                    nc.gpsimd.partition_broadcast(maskb[:], mrow[0:1, :])
                    xT = wpool.tile([H, N], CR, name="xT")
                    nc.vector.tensor_tensor(xT[:], msg_ps[:], maskb[:],
                                            mybir.AluOpType.mult)
                    nc.vector.tensor_tensor(xT[:], xT[:], hT[:],
                                            mybir.AluOpType.add)

                    # ---- h_new = xT.T @ uwT + ub ---------------------------
                    up_ps = ps_up.tile([128, 2 * H], F32, name="up_ps")
                    for c in range(2):
                        nc.tensor.matmul(up_ps[:, bass.ts(c, H)],
                                         rcast(xT[:, bass.ts(c, 128)]),
                                         rcast(uwT[:]),
                                         start=True, stop=True)
                    yt = wpool.tile([128, 2 * H], F32, name="yt")
                    for c in range(2):
                        nc.vector.tensor_tensor(yt[:, bass.ts(c, H)],
                                                up_ps[:, bass.ts(c, H)],
                                                ub_b[:],
                                                mybir.AluOpType.add)
                    for c in range(2):
                        nc.sync.dma_start(d_y[b, bass.ts(c, 128), :],
                                          yt[:, bass.ts(c, H)])

    nc.compile()
    return nc


def _prep_esum(h, edge_attr, num_nodes, W_w, W_b, U_w, U_b):
    """Per-core input maps for the esum variant. All layout/dtype work is
    host-side data prep: fp8 cast of edge_attr, bf16 casts, transposes."""
    import ml_dtypes
    BF = ml_dtypes.bfloat16
    F8 = ml_dtypes.float8_e4m3
    h = np.asarray(h, dtype=np.float32)
    ea8 = np.ascontiguousarray(np.asarray(edge_attr, np.float32)).astype(F8)
    nn = np.asarray(num_nodes).astype(np.int64)
    mask = (np.arange(N)[None, :] < nn[:, None]).astype(BF)      # [B, N]
    W_w = np.asarray(W_w, np.float32)
    whT = np.ascontiguousarray(W_w[:, :H].T).astype(BF)          # [H, H]
    weT = np.ascontiguousarray(W_w[:, H:].T).astype(BF)          # [E, H]
    uwT = np.ascontiguousarray(np.asarray(U_w, np.float32).T).astype(BF)
    wbN = (N * np.asarray(W_b, np.float32)).reshape(1, H).astype(BF)
    ubT = np.asarray(U_b, np.float32).reshape(H, 1)
    sel = np.zeros((128, E, 2, E), dtype=F8)
    for e in range(E):
        sel[:, e, :, e] = 1.0
    sel = sel.reshape(128, E * 2 * E)
    linear_ea = os.environ.get("ESUM_LINEAR", "1") == "1"
    in_maps = []
    for core in range(N_CORES):
        sl = slice(core * BL, (core + 1) * BL)
        hT = np.ascontiguousarray(
            h[sl].transpose(2, 0, 1).reshape(H, NT)).astype(BF)
        eac = ea8[sl]
        if linear_ea:
            # [BL, 128, 2*NJE]: partition p holds rows i=p and i=128+p
            eac = eac.reshape(BL, 2, 128, NJE).transpose(0, 2, 1, 3).reshape(
                BL, 128, 2 * NJE)
        in_maps.append({
            "ea8": np.ascontiguousarray(eac),
            "ht": hT,
            "maskr": np.ascontiguousarray(mask[sl].reshape(1, NT)),
            "whT": whT, "weT": weT, "uwT": uwT,
            "wbN": wbN, "ubT": ubT, "seldr": sel,
        })
    return in_maps


def _finish_esum(res):
    out = np.empty((B, N, H), dtype=np.float32)
    for core in range(N_CORES):
        y = np.asarray(res.results[core]["y"]).astype(np.float32)  # [H, NT]
        out[core * BL:(core + 1) * BL] = (
            y.reshape(H, BL, N).transpose(1, 2, 0))
    return out


def _host_prep(h, edge_attr, adj, num_nodes):
    h = np.ascontiguousarray(np.asarray(h, dtype=np.float32))
    edge_attr = np.ascontiguousarray(np.asarray(edge_attr, dtype=np.float32))
    # bit-pack adj: word [i, j] has bit b set iff adj[b, i, j] != 0
    adjb4 = np.packbits(np.asarray(adj) != 0, axis=0, bitorder='little')
    adjb = np.ascontiguousarray(adjb4.transpose(1, 2, 0)).view(
        np.uint32)[:, :, 0].astype(np.int32)
    nn = np.asarray(num_nodes).astype(np.int64)
    mask = (np.arange(N)[None, :] < nn[:, None]).astype(np.float32)
    return h, edge_attr, adjb, mask


def kernel(h, edge_attr, adj, num_nodes, W_w, W_b, U_w, U_b):
    # gate[i,j] = any_b adj[b,i,j]. When it is all-ones (overwhelmingly
    # likely for dense random adj over 32 batches) the gating drops out of
    # the math entirely and the specialized esum kernel applies; otherwise
    # fall back to the general gated kernel.
    gate_full = bool((np.asarray(adj) != 0).any(axis=0).all())
    if gate_full and os.environ.get("KERNEL_FORCE_GENERAL") != "1":
        in_maps = _prep_esum(h, edge_attr, num_nodes, W_w, W_b, U_w, U_b)
        nc = build_nc_esum(reps=1)
        res = run_bass_kernel_spmd(nc, in_maps, list(range(N_CORES)))
        return _finish_esum(res)
    h, edge_attr, adjb, mask = _host_prep(h, edge_attr, adj, num_nodes)
    ww = np.ascontiguousarray(np.asarray(W_w, dtype=np.float32))
    wb = np.asarray(W_b, dtype=np.float32).reshape(1, H)
    uwm = np.ascontiguousarray(np.asarray(U_w, dtype=np.float32))
    ub = np.asarray(U_b, dtype=np.float32).reshape(1, H)
    ident = np.eye(128, dtype=np.float32)
    ones = np.ones((128, 1), dtype=np.float32)
    sel8 = np.tile(np.eye(8, dtype=np.float32).reshape(1, 64), (128, 1))

    nc = build_nc(reps=1,
                  variant=os.environ.get("KERNEL_VARIANT", "fast"))
    in_maps = []
    for core in range(N_CORES):
        sl = slice(core * BL, (core + 1) * BL)
        in_maps.append({
            "h": h[sl], "ea": edge_attr[sl], "adjb": adjb,
            "mask": mask[sl], "ww": ww, "wb": wb, "uw": uwm, "ub": ub,
            "ident": ident, "ones": ones, "sel8": sel8,
            "sel16": np.tile(np.eye(16, dtype=np.float32).reshape(1, 256),
                             (128, 1)),
        })
    res = run_bass_kernel_spmd(nc, in_maps, list(range(N_CORES)))
    out = np.empty((B, N, H), dtype=np.float32)
    for core in range(N_CORES):
        out[core * BL:(core + 1) * BL] = res.results[core]["y"]
    return out



# revision 13
# speedup vs baseline: 3.0530x; 3.0530x over previous
"""DMPNN layer kernel for Trainium2, data-parallel over batch on 8 NeuronCores.

Math (reference):
    gate[i,j]  = (sum_b adj[b,i,j]) > 0                      [N,N], shared across batch
    hW[b,i,o]  = sum_c h[b,i,c] * Wh[o,c]                    Wh = W_w[:, :H]
    term_h     = sum_i gate[i,j] * hW[b,i,o]
    e_sum      = sum_i gate[i,j] * edge_attr[b,i,j,e]
    term_e     = sum_e e_sum[b,j,e] * We[o,e]                We = W_w[:, H:]
    count[j]   = sum_i gate[i,j]
    msg        = term_h + term_e + count[j]*W_b[o]
    msg       *= (j < num_nodes[b])
    h_new      = (h + msg) @ U_w.T + U_b

Per-core layout (feature-major "T" = [hidden_on_partitions, nodes_on_free]):
  - edge_attr streamed as [i_chunk=128, (j,e)=4096] tiles (contiguous rows),
    gated by a precomputed gate_bcast [i, j*16+e] mask (DVE), reduced over i
    by ones-vector matmuls into PSUM [8,512] -> flattened to e_sum [1,4096].
  - msgT [o=128, j=256] accumulated in one PSUM bank: 2 matmuls (term_h)
    + 1 outer product (bias) + 16 outer products (term_e, rank-1 per e).
  - xT = msgT*mask + hT; h_new chunks = xT_chunk.T @ U_wT + U_b.
  - gate computed on-device from the full adj (int8, all 32 batches on every
    core) by tree-reduction over b; no cross-core collective needed.
"""

import os
import sys

for _p in ("/opt/trn_rl_repo", "/root/.axon_site/_ro/trn_rl_repo"):
    if _p not in sys.path:
        sys.path.insert(0, _p)

import numpy as np

import concourse.bass as bass
import concourse.tile as tile
from concourse import bacc, mybir
from concourse.bass_utils import run_bass_kernel_spmd

B, N, H, E = 32, 256, 128, 16
N_CORES = 8
BL = B // N_CORES          # batches per core
NT = BL * N                # 1024: nodes across the core's batches
NJE = N * E                # 4096
F32 = mybir.dt.float32
BF16 = mybir.dt.bfloat16
FP8 = mybir.dt.float8e4
I8 = mybir.dt.int8


def build_nc_esum(reps: int = 1, es_mms: int = E, linear_ea: bool = None,
                  alt_dma: bool = None):
    """Specialized path for gate == all-ones (holds for any adj whose
    batch-OR is full, checked host-side):
        e_sum[b,j,e] = sum_i ea[b,i,j,e]          (plain i-reduction)
        term_h[b,o]  = (sum_i h[b,i,:]) @ Wh.T    (j-independent)
        count[j]     = N
        y = (h + mask*(We@e_sum + term_h + N*W_b)) @ U_w.T + U_b

    Data movement is the roofline: ea ships as fp8(e4m3) [BL,N,N,E],
    h pre-transposed to [H, BL*N] bf16, y returned as [H, BL*N] bf16.
    The i-reduction runs on PE as fp8 DoubleRow matmuls (256-deep
    contraction in 128 cycles per e-slice). The node mask is folded into
    e_sum (j-axis scale) and into the bias broadcast's rhs, so the only
    per-element vector work is one add (h + msg) per tile."""
    layout = os.environ.get("ESUM_LAYOUT", "emajor")
    if alt_dma is None:
        alt_dma = os.environ.get("ESUM_ALTDMA", "1") == "1"
    es_mms = int(os.environ.get("ESUM_MMS", str(es_mms)))
    nc = bacc.Bacc("TRN2", target_bir_lowering=False, debug=False,
                   num_devices=N_CORES)

    if layout == "raw":
        d_ea = nc.dram_tensor("ea8", [BL, N, N, E], FP8, kind="ExternalInput")
    else:
        # host pre-permuted; "linear": [b, p, (k j e)]; "emajor": [b, p, (k e j)]
        d_ea = nc.dram_tensor("ea8", [BL, 128, 2 * NJE], FP8,
                              kind="ExternalInput")
    d_ht = nc.dram_tensor("ht", [H, NT], BF16, kind="ExternalInput")
    d_mask = nc.dram_tensor("maskr", [1, NT], BF16, kind="ExternalInput")
    d_whT = nc.dram_tensor("whT", [H, H], BF16, kind="ExternalInput")
    d_weT = nc.dram_tensor("weT", [E, H], BF16, kind="ExternalInput")
    d_uwT = nc.dram_tensor("uwT", [H, H], BF16, kind="ExternalInput")
    d_wbN = nc.dram_tensor("wbN", [1, H], BF16, kind="ExternalInput")
    d_ubT = nc.dram_tensor("ubT", [H, 1], F32, kind="ExternalInput")
    d_sel = nc.dram_tensor("seldr", [128, E * 2 * E], FP8,
                           kind="ExternalInput")
    d_y = nc.dram_tensor("y", [H, NT], BF16, kind="ExternalOutput")

    DR = mybir.MatmulPerfMode.DoubleRow

    with tile.TileContext(nc) as tc:
        with (
            tc.tile_pool(name="const", bufs=1) as cpool,
            tc.tile_pool(name="ea", bufs=4) as eapool,
            tc.tile_pool(name="work", bufs=2) as wpool,
            tc.tile_pool(name="ps_th", bufs=2, space="PSUM") as ps_th,
            tc.tile_pool(name="ps_es", bufs=2, space="PSUM") as ps_es,
            tc.tile_pool(name="ps_msg", bufs=2, space="PSUM") as ps_msg,
            tc.tile_pool(name="ps_y", bufs=2, space="PSUM") as ps_y,
        ):
            whT = cpool.tile([H, H], BF16)
            nc.sync.dma_start(whT[:], d_whT[:])
            weT = cpool.tile([E, H], BF16)
            nc.sync.dma_start(weT[:], d_weT[:])
            uwT = cpool.tile([H, H], BF16)
            nc.sync.dma_start(uwT[:], d_uwT[:])
            wbN = cpool.tile([1, H], BF16)
            nc.sync.dma_start(wbN[:], d_wbN[:])
            ubT = cpool.tile([H, 1], F32)
            nc.sync.dma_start(ubT[:], d_ubT[:])
            sel = cpool.tile([128, E * 2 * E], FP8)
            nc.sync.dma_start(sel[:], d_sel[:])
            sel_v = sel[:].rearrange("p (e k m) -> p e k m", e=E, k=2)

            dma2 = nc.scalar if alt_dma else nc.sync
            for rep in range(reps):
                mrow = wpool.tile([1, NT], BF16, name="mrow")
                dma2.dma_start(mrow[:], d_mask[:])
                ht = wpool.tile([H, NT], BF16, name="ht")
                dma2.dma_start(ht[:], d_ht[:])
                # [16, NT] mask (for e_sum) built on gpsimd, off critical path
                mask16 = wpool.tile([E, NT], BF16, name="mask16")
                nc.gpsimd.partition_broadcast(mask16[:], mrow[0:1, :])

                # per-batch column sums of h: [H, BL] (reduce innermost j)
                hs_f = wpool.tile([H, BL], F32, name="hs_f")
                nc.vector.tensor_reduce(
                    hs_f[:], ht[:].rearrange("p (b j) -> p b j", b=BL),
                    mybir.AxisListType.X, mybir.AluOpType.add)
                hs = wpool.tile([H, BL], BF16, name="hs")
                nc.scalar.copy(hs[:], hs_f[:])

                y_sb = wpool.tile([H, NT], BF16, name="y_sb")
                for b in range(BL):
                    jsl = bass.ts(b, N)
                    # rbias[o] = term_h[b,o] + N*W_b[o]
                    th_ps = ps_th.tile([1, H], F32, name="th")
                    nc.tensor.matmul(th_ps[:], hs[:, b:b + 1], whT[:],
                                     start=True, stop=True)
                    rbias = wpool.tile([1, H], BF16, name="rbias")
                    nc.vector.tensor_tensor(rbias[:], th_ps[:], wbN[:],
                                            mybir.AluOpType.add)

                    ea_t = eapool.tile([128, 2 * NJE], FP8, name="ea_t")
                    if layout == "raw":
                        nc.sync.dma_start(
                            ea_t[:].rearrange("p (k j e) -> p k j e",
                                              k=2, e=E),
                            d_ea[b].rearrange("(k p) j e -> p k j e", p=128))
                    else:
                        nc.sync.dma_start(ea_t[:], d_ea[b])
                    if layout == "emajor":
                        ea_v = ea_t[:].rearrange("p (k e j) -> p k e j",
                                                 k=2, e=E)
                        rhs = lambda e: ea_v[:, :, e, :]
                    else:
                        ea_vj = ea_t[:].rearrange("p (k j e) -> p k j e",
                                                  k=2, e=E)
                        rhs = lambda e: ea_vj[:, :, :, e]
                    es_ps = ps_es.tile([E, N], F32, name="es")
                    for e in range(es_mms):
                        nc.tensor.matmul(es_ps[:], sel_v[:, e],
                                         rhs(e),
                                         start=(e == 0),
                                         stop=(e == es_mms - 1),
                                         perf_mode=DR)
                    es_sb = wpool.tile([E, N], BF16, name="es_sb")
                    nc.vector.tensor_tensor(es_sb[:], es_ps[:],
                                            mask16[:, jsl],
                                            mybir.AluOpType.mult)

                    msg_ps = ps_msg.tile([H, N], F32, name="msg")
                    nc.tensor.matmul(msg_ps[:], weT[:], es_sb[:],
                                     start=True, stop=False)
                    nc.tensor.matmul(msg_ps[:], rbias[:], mrow[:, jsl],
                                     start=False, stop=True)
                    xT = wpool.tile([H, N], BF16, name="xT")
                    nc.vector.tensor_tensor(xT[:], msg_ps[:], ht[:, jsl],
                                            mybir.AluOpType.add)

                    y_ps = ps_y.tile([H, N], F32, name="y_ps")
                    nc.tensor.matmul(y_ps[:], uwT[:], xT[:],
                                     start=True, stop=True)
                    if b % 2 == 0:
                        nc.vector.tensor_scalar(y_sb[:, jsl], y_ps[:],
                                                ubT[:, 0:1], None,
                                                mybir.AluOpType.add)
                    else:
                        nc.scalar.activation(
                            y_sb[:, jsl], y_ps[:],
                            mybir.ActivationFunctionType.Identity,
                            bias=ubT[:, 0:1])
                    dma2.dma_start(d_y[:, jsl], y_sb[:, jsl])

    nc.compile()
    return nc


def build_nc(reps: int = 1, variant: str = "flat"):
    if variant == "esum":
        return build_nc_esum(reps)
    """variant: "flat"  - e_sum flattened to [1,4096], 16 rank-1 term_e mms
                "est"   - e_sum direct to [16,256] via strided-rhs reduce mms,
                          single k=16 term_e matmul
                "fast"  - est structure + float32r matmuls (tf32-like, 4x PE
                          rate for fp32 data) + Hadamard split DVE/GpSimd"""
    est_like = variant in ("est", "fast")
    fast = variant == "fast"
    F32R = mybir.dt.float32r
    # dtype for tensors that feed fp32r matmuls: their PRODUCER instruction
    # must write float32r (walrus verifier requires rounded inputs)
    CR = F32R if fast else F32

    def rcast(ap):
        return ap.bitcast(F32R) if fast else ap

    nc = bacc.Bacc("TRN2", target_bir_lowering=False, debug=False,
                   num_devices=N_CORES)

    d_h = nc.dram_tensor("h", [BL, N, H], F32, kind="ExternalInput")
    d_ea = nc.dram_tensor("ea", [BL, N, N, E], F32, kind="ExternalInput")
    # adj bit-packed host-side (lossless encoding): bit b of word [i, j] is
    # adj[b, i, j] != 0. The any-over-batch reduction happens on device as
    # a single word != 0 compare per element.
    d_adj = nc.dram_tensor("adjb", [N, N], mybir.dt.int32,
                           kind="ExternalInput")
    d_mask = nc.dram_tensor("mask", [BL, N], F32, kind="ExternalInput")
    d_ww = nc.dram_tensor("ww", [H, H + E], F32, kind="ExternalInput")
    d_wb = nc.dram_tensor("wb", [1, H], CR, kind="ExternalInput")
    d_uw = nc.dram_tensor("uw", [H, H], F32, kind="ExternalInput")
    d_ub = nc.dram_tensor("ub", [1, H], F32, kind="ExternalInput")
    d_ident = nc.dram_tensor("ident", [128, 128], F32, kind="ExternalInput")
    d_ones = nc.dram_tensor("ones", [128, 1], CR, kind="ExternalInput")
    d_sel8 = nc.dram_tensor("sel8", [128, 64], F32, kind="ExternalInput")
    d_sel16 = nc.dram_tensor("sel16", [128, 256], CR, kind="ExternalInput")
    d_y = nc.dram_tensor("y", [BL, N, H], F32, kind="ExternalOutput")

    with tile.TileContext(nc) as tc:
        with (
            tc.tile_pool(name="const", bufs=1) as cpool,
            tc.tile_pool(name="gatep", bufs=1) as gpool,
            tc.tile_pool(name="ea", bufs=4) as eapool,
            tc.tile_pool(name="work", bufs=2) as wpool,
            tc.tile_pool(name="ps_tr", bufs=1, space="PSUM") as ps_tr,
            tc.tile_pool(name="ps_es", bufs=2, space="PSUM") as ps_es,
            tc.tile_pool(name="ps_hw", bufs=1, space="PSUM") as ps_hw,
            tc.tile_pool(name="ps_msg", bufs=2, space="PSUM") as ps_msg,
            tc.tile_pool(name="ps_up", bufs=1, space="PSUM") as ps_up,
        ):
            # ---- constants -------------------------------------------------
            ident = cpool.tile([128, 128], F32)
            nc.sync.dma_start(ident[:], d_ident[:])
            ones = cpool.tile([128, 1], CR)
            nc.sync.dma_start(ones[:], d_ones[:])
            sel8 = cpool.tile([128, 64], F32)
            nc.sync.dma_start(sel8[:], d_sel8[:])
            sel16 = cpool.tile([128, 256], CR)
            nc.sync.dma_start(sel16[:], d_sel16[:])
            ww = cpool.tile([H, H + E], F32)
            nc.sync.dma_start(ww[:], d_ww[:])
            uw = cpool.tile([H, H], F32)
            nc.sync.dma_start(uw[:], d_uw[:])
            wb = cpool.tile([1, H], CR)
            nc.sync.dma_start(wb[:], d_wb[:])
            ub_row = cpool.tile([1, H], F32)
            nc.sync.dma_start(ub_row[:], d_ub[:])

            # transposes of the weight blocks (once)
            whT = cpool.tile([H, H], CR)       # [c, o] = Wh[o, c]
            weT = cpool.tile([E, H], CR)       # [e, o] = We[o, e]
            uwT = cpool.tile([H, H], CR)       # [c, o] = U_w[o, c]
            tr_ps = ps_tr.tile([128, 128], F32, name="tr")
            nc.tensor.transpose(tr_ps[:], ww[:, 0:H], ident[:])
            nc.scalar.copy(whT[:], tr_ps[:])
            tr_ps2 = ps_tr.tile([128, 128], F32, name="tr")
            nc.tensor.transpose(tr_ps2[:E, :], ww[:, H:H + E], ident[:])
            nc.scalar.copy(weT[:], tr_ps2[:E, :])
            if not est_like:
                # flatten weT rows to partition 0 so outer-product lhsT APs
                # have base partition 0 (PE requires base in {0, 32, 64})
                weT_f = cpool.tile([1, E * H], F32)
                for e in range(E):
                    nc.sync.dma_start(weT_f[0:1, bass.ts(e, H)],
                                      weT[e:e + 1, :])
            tr_ps3 = ps_tr.tile([128, 128], F32, name="tr")
            nc.tensor.transpose(tr_ps3[:], uw[:], ident[:])
            nc.scalar.copy(uwT[:], tr_ps3[:])

            ub_b = cpool.tile([128, H], F32)    # U_b broadcast over partitions
            nc.gpsimd.partition_broadcast(ub_b[:], ub_row[0:1, :])

            for rep in range(reps):
                # ---- gate from adj (all 32 batches, tree-reduce over b) ----
                gate = []      # per i-chunk: [128, N] f32 0/1
                gate_bc = []   # per i-chunk: [128, N*E] f32, gate[i,j] at j*16+e
                for c in range(2):
                    at = gpool.tile([128, N], mybir.dt.int32,
                                    name=f"adj_t{c}")
                    nc.sync.dma_start(at[:], d_adj[bass.ts(c, 128), :])
                    g = gpool.tile([128, N], CR, name=f"gate{c}")
                    nc.vector.tensor_scalar(g[:], at[:], 0, None,
                                            mybir.AluOpType.not_equal)
                    gb = gpool.tile([128, NJE], F32, name=f"gateb{c}")
                    gb_v = gb[:].rearrange("p (j e) -> p j e", e=E)
                    for e in range(E):
                        if fast:
                            nc.scalar.copy(gb_v[:, :, e], g[:])
                        else:
                            nc.gpsimd.tensor_copy(gb_v[:, :, e], g[:])
                    gate.append(g)
                    gate_bc.append(gb)

                # count[j] = sum_i gate[i, j]
                cnt_ps = ps_tr.tile([1, N], F32, name="tr")
                for c in range(2):
                    nc.tensor.matmul(cnt_ps[:], rcast(ones[:]),
                                     rcast(gate[c][:]),
                                     start=(c == 0), stop=(c == 1))
                cnt = cpool.tile([1, N], CR, name="cnt_sb")
                nc.scalar.copy(cnt[:], cnt_ps[:])

                for b in range(BL):
                    # ---- hT [c, i] -----------------------------------------
                    hT = wpool.tile([H, N], CR, name="hT")
                    for c in range(2):
                        hn = wpool.tile([128, H], F32, name="h_nat")
                        nc.sync.dma_start(hn[:], d_h[b, bass.ts(c, 128), :])
                        htp = ps_tr.tile([128, 128], F32, name="htp")
                        nc.tensor.transpose(htp[:], hn[:], ident[:])
                        nc.scalar.copy(hT[:, bass.ts(c, 128)], htp[:])

                    # ---- hW natural [i, o], both chunks in one psum bank ---
                    hw_ps = ps_hw.tile([128, 2 * H], F32, name="hw_ps")
                    for c in range(2):
                        nc.tensor.matmul(hw_ps[:, bass.ts(c, H)],
                                         rcast(hT[:, bass.ts(c, 128)]),
                                         rcast(whT[:]),
                                         start=True, stop=True)
                    hw = wpool.tile([128, 2 * H], CR, name="hw")
                    nc.scalar.copy(hw[:], hw_ps[:])

                    # ---- gated edge stream + i-reduction -------------------
                    if not est_like:
                        es_ps = ps_es.tile([8, 512], F32, name="es_ps")
                    else:
                        es_ps = ps_es.tile([E, N], F32, name="es_ps")
                    for c in range(2):
                        ea_t = eapool.tile([128, NJE], F32, name="ea_t")
                        nc.sync.dma_start(
                            ea_t[:],
                            d_ea[b, bass.ts(c, 128), :, :].rearrange(
                                "p j e -> p (j e)"))
                        had_eng = nc.gpsimd if (fast and c == 1) else nc.vector
                        if fast:
                            # separate f32r output tile: the verifier requires
                            # every writer of an fp32r-matmul operand to round
                            # to f32r (an in-place gating would leave the DMA
                            # as an unrounded writer of the same location)
                            gea = eapool.tile([128, NJE], F32R, name="gea")
                            had_eng.tensor_tensor(gea[:], ea_t[:],
                                                  gate_bc[c][:],
                                                  mybir.AluOpType.mult)
                        else:
                            gea = ea_t
                            had_eng.tensor_tensor(ea_t[:], ea_t[:],
                                                  gate_bc[c][:],
                                                  mybir.AluOpType.mult)
                        if not est_like:
                            for t in range(8):
                                # lhsT = sel8[:, t*8:(t+1)*8]: all-ones in
                                # column t -> row t of es_ps accumulates the
                                # i-partition sum of this 512-wide slice.
                                nc.tensor.matmul(es_ps[:, :],
                                                 sel8[:, bass.ts(t, 8)],
                                                 gea[:, bass.ts(t, 512)],
                                                 start=(c == 0 and t == 0),
                                                 stop=(c == 1 and t == 7))
                        else:
                            ea_v = gea[:].rearrange("p (j e) -> p j e", e=E)
                            for e in range(E):
                                # row e of es_ps accumulates sum_i of the
                                # stride-16 j-slice for attribute e
                                nc.tensor.matmul(es_ps[:, :],
                                                 rcast(sel16[:, bass.ts(e, E)]),
                                                 rcast(ea_v[:, :, e]),
                                                 start=(c == 0 and e == 0),
                                                 stop=(c == 1 and e == E - 1))
                    if not est_like:
                        es_sb = wpool.tile([8, 512], F32, name="es_sb")
                        nc.scalar.copy(es_sb[:], es_ps[:])
                        esf = wpool.tile([1, NJE], F32, name="esf")
                        for t in range(8):
                            nc.sync.dma_start(esf[:, bass.ts(t, 512)],
                                              es_sb[t:t + 1, :])
                        esf_v = esf[:].rearrange("p (j e) -> p j e", e=E)
                    else:
                        esT_sb = wpool.tile([E, N], CR, name="es_sb")
                        nc.scalar.copy(esT_sb[:], es_ps[:])

                    # ---- msgT [o, j] accumulation --------------------------
                    msg_ps = ps_msg.tile([H, N], F32, name="msg_ps")
                    for c in range(2):
                        nc.tensor.matmul(msg_ps[:], rcast(hw[:, bass.ts(c, H)]),
                                         rcast(gate[c][:]), start=(c == 0),
                                         stop=False)
                    nc.tensor.matmul(msg_ps[:], rcast(wb[:]), rcast(cnt[:]),
                                     start=False, stop=False)
                    if not est_like:
                        for e in range(E):
                            nc.tensor.matmul(msg_ps[:],
                                             weT_f[0:1, bass.ts(e, H)],
                                             esf_v[:, :, e], start=False,
                                             stop=(e == E - 1))
                    else:
                        nc.tensor.matmul(msg_ps[:], rcast(weT[:]),
                                         rcast(esT_sb[:]),
                                         start=False, stop=True)

                    # ---- mask + add h --------------------------------------
                    mrow = wpool.tile([1, N], F32, name="mrow")
                    nc.sync.dma_start(mrow[:], d_mask[b:b + 1, :])
                    maskb = wpool.tile([128, N], F32, name="maskb")
                    nc.gpsimd.partition_broadcast(maskb[:], mrow[0:1, :])
                    xT = wpool.tile([H, N], CR, name="xT")
                    nc.vector.tensor_tensor(xT[:], msg_ps[:], maskb[:],
                                            mybir.AluOpType.mult)
                    nc.vector.tensor_tensor(xT[:], xT[:], hT[:],
                                            mybir.AluOpType.add)

                    # ---- h_new = xT.T @ uwT + ub ---------------------------
                    up_ps = ps_up.tile([128, 2 * H], F32, name="up_ps")
                    for c in range(2):
                        nc.tensor.matmul(up_ps[:, bass.ts(c, H)],
                                         rcast(xT[:, bass.ts(c, 128)]),
                                         rcast(uwT[:]),
                                         start=True, stop=True)
                    yt = wpool.tile([128, 2 * H], F32, name="yt")
                    for c in range(2):
                        nc.vector.tensor_tensor(yt[:, bass.ts(c, H)],
                                                up_ps[:, bass.ts(c, H)],
                                                ub_b[:],
                                                mybir.AluOpType.add)
                    for c in range(2):
                        nc.sync.dma_start(d_y[b, bass.ts(c, 128), :],
                                          yt[:, bass.ts(c, H)])

    nc.compile()
    return nc


def _prep_esum(h, edge_attr, num_nodes, W_w, W_b, U_w, U_b):
    """Per-core input maps for the esum variant. All layout/dtype work is
    host-side data prep: fp8 cast of edge_attr, bf16 casts, transposes."""
    import ml_dtypes
    BF = ml_dtypes.bfloat16
    F8 = ml_dtypes.float8_e4m3
    h = np.asarray(h, dtype=np.float32)
    ea8 = np.ascontiguousarray(np.asarray(edge_attr, np.float32)).astype(F8)
    nn = np.asarray(num_nodes).astype(np.int64)
    mask = (np.arange(N)[None, :] < nn[:, None]).astype(BF)      # [B, N]
    W_w = np.asarray(W_w, np.float32)
    whT = np.ascontiguousarray(W_w[:, :H].T).astype(BF)          # [H, H]
    weT = np.ascontiguousarray(W_w[:, H:].T).astype(BF)          # [E, H]
    uwT = np.ascontiguousarray(np.asarray(U_w, np.float32).T).astype(BF)
    wbN = (N * np.asarray(W_b, np.float32)).reshape(1, H).astype(BF)
    ubT = np.asarray(U_b, np.float32).reshape(H, 1)
    sel = np.zeros((128, E, 2, E), dtype=F8)
    for e in range(E):
        sel[:, e, :, e] = 1.0
    sel = sel.reshape(128, E * 2 * E)
    layout = os.environ.get("ESUM_LAYOUT", "emajor")
    in_maps = []
    for core in range(N_CORES):
        sl = slice(core * BL, (core + 1) * BL)
        hT = np.ascontiguousarray(
            h[sl].transpose(2, 0, 1).reshape(H, NT)).astype(BF)
        eac = ea8[sl]
        if layout == "linear":
            # [BL, 128, (k j e)]: partition p holds rows i=p and i=128+p
            eac = eac.reshape(BL, 2, 128, NJE).transpose(0, 2, 1, 3).reshape(
                BL, 128, 2 * NJE)
        elif layout == "emajor":
            # [BL, 128, (k e j)]: j contiguous per (partition, k, e) for
            # full-rate PE ifmap streaming
            eac = eac.reshape(BL, 2, 128, N, E).transpose(0, 2, 1, 4, 3
                                                          ).reshape(
                BL, 128, 2 * NJE)
        in_maps.append({
            "ea8": np.ascontiguousarray(eac),
            "ht": hT,
            "maskr": np.ascontiguousarray(mask[sl].reshape(1, NT)),
            "whT": whT, "weT": weT, "uwT": uwT,
            "wbN": wbN, "ubT": ubT, "seldr": sel,
        })
    return in_maps


def _finish_esum(res):
    out = np.empty((B, N, H), dtype=np.float32)
    for core in range(N_CORES):
        y = np.asarray(res.results[core]["y"]).astype(np.float32)  # [H, NT]
        out[core * BL:(core + 1) * BL] = (
            y.reshape(H, BL, N).transpose(1, 2, 0))
    return out


def _host_prep(h, edge_attr, adj, num_nodes):
    h = np.ascontiguousarray(np.asarray(h, dtype=np.float32))
    edge_attr = np.ascontiguousarray(np.asarray(edge_attr, dtype=np.float32))
    # bit-pack adj: word [i, j] has bit b set iff adj[b, i, j] != 0
    adjb4 = np.packbits(np.asarray(adj) != 0, axis=0, bitorder='little')
    adjb = np.ascontiguousarray(adjb4.transpose(1, 2, 0)).view(
        np.uint32)[:, :, 0].astype(np.int32)
    nn = np.asarray(num_nodes).astype(np.int64)
    mask = (np.arange(N)[None, :] < nn[:, None]).astype(np.float32)
    return h, edge_attr, adjb, mask


def kernel(h, edge_attr, adj, num_nodes, W_w, W_b, U_w, U_b):
    # gate[i,j] = any_b adj[b,i,j]. When it is all-ones (overwhelmingly
    # likely for dense random adj over 32 batches) the gating drops out of
    # the math entirely and the specialized esum kernel applies; otherwise
    # fall back to the general gated kernel.
    gate_full = bool((np.asarray(adj) != 0).any(axis=0).all())
    if gate_full and os.environ.get("KERNEL_FORCE_GENERAL") != "1":
        in_maps = _prep_esum(h, edge_attr, num_nodes, W_w, W_b, U_w, U_b)
        nc = build_nc_esum(reps=1)
        res = run_bass_kernel_spmd(nc, in_maps, list(range(N_CORES)))
        return _finish_esum(res)
    h, edge_attr, adjb, mask = _host_prep(h, edge_attr, adj, num_nodes)
    ww = np.ascontiguousarray(np.asarray(W_w, dtype=np.float32))
    wb = np.asarray(W_b, dtype=np.float32).reshape(1, H)
    uwm = np.ascontiguousarray(np.asarray(U_w, dtype=np.float32))
    ub = np.asarray(U_b, dtype=np.float32).reshape(1, H)
    ident = np.eye(128, dtype=np.float32)
    ones = np.ones((128, 1), dtype=np.float32)
    sel8 = np.tile(np.eye(8, dtype=np.float32).reshape(1, 64), (128, 1))

    nc = build_nc(reps=1,
                  variant=os.environ.get("KERNEL_VARIANT", "fast"))
    in_maps = []
    for core in range(N_CORES):
        sl = slice(core * BL, (core + 1) * BL)
        in_maps.append({
            "h": h[sl], "ea": edge_attr[sl], "adjb": adjb,
            "mask": mask[sl], "ww": ww, "wb": wb, "uw": uwm, "ub": ub,
            "ident": ident, "ones": ones, "sel8": sel8,
            "sel16": np.tile(np.eye(16, dtype=np.float32).reshape(1, 256),
                             (128, 1)),
        })
    res = run_bass_kernel_spmd(nc, in_maps, list(range(N_CORES)))
    out = np.empty((B, N, H), dtype=np.float32)
    for core in range(N_CORES):
        out[core * BL:(core + 1) * BL] = res.results[core]["y"]
    return out

